# revision 26
# baseline (speedup 1.0000x reference)
"""AttnBlock (GroupNorm + 4-head d=128 self-attention + residual).

Full input x: [8, 512, 2048] fp32. Data-parallel over batch: core b computes
batch b entirely on-chip (no collectives).

v2 design (per-core, C=512, L=2048, G=4 groups, NH=4 heads, HD=128):
  h  = groupnorm(x)            fp8 (rstd via DVE Newton rsqrt -- no ACT Sqrt,
                               so the Exp table is loaded once and never again)
  k/q = w @ h + b              bf16, fp8 DoubleRow matmuls (weights x16)
  vT  = h^T @ wv^T + bv        fp8 [l, c]
  sT[k,q] = k_tile^T q         fp32 PSUM (bf16 operands)
  e = exp(s*scale - 3)         one ACT instr per [128,1024] group -> fp8
                               (shift keeps e in fp8e4 range; softmax is
                               shift-invariant)
  den: fp8 DoubleRow ones-matmul accumulated on the PE (no DVE tree);
       rden = reciprocal_approx_fast; attn = pav * rden (bf16)
  out = wo @ attn + bo + x     bf16 matmuls, DVE STT fuses bias + residual

ACT is the pacer: 128 exps x ~1.11us. Everything else hides under it.
GPSIMD is avoided entirely for compute: its tensor ops run ~13x slower
than DVE on hardware and stall concurrent DVE instructions.

Pipeline (uniform lag): pden/pav of unit U accumulate on the PE during
U+1 (2 cheap DR matmuls per hook over the persistent e tiles, epool=16);
finish(U) (reciprocal + normalize) runs at U+2's g2 hook. PSUM banks:
scores sA/sB (4) + dn0/dn1 + av0/av1: during unit u, dn/av[(u+1)%2]
receive U-1's accumulation while dn/av[u%2] are finished at g2 and then
serve as transient banks (proj drains, out-proj) at hooks g>=3.

DMA: the dispatching sequencer blocks when its hwdge ring fills, so qACT
carries only the x b-halves (ACT must stay free for applies + exps); qSP
carries cvecs, x a-halves, wk/wq block-0 mid-stream, wv01 and the late
stages + all weight DMA-transposes (XBAR, zero PE cost); the GPSIMD SWDGE
queue runs a second weight stream (wv23, bv, ...).
x-ct3 lands in staggered quarters so its bn_stats pipeline behind the
stream. Weight path: stage f32 -> DVE bf16 cast (x16) -> qSP DMA
transpose -> DVE fp8 cast (wo stops at bf16, transposed in place).
Unit order delays head 1 to idx 3 so projections ride 1-2 units ahead
of first use; qc2's last unit sits at idx13 so its out-projections ride
idx15's hooks; qc3 is the tail (4-way parallel output DMAs).
"""

import os
import numpy as np

import concourse.bass as bass
import concourse.tile as tile
from concourse import bacc, mybir
from concourse.bass_utils import run_bass_kernel_spmd
from concourse.masks import make_identity

F32 = mybir.dt.float32
BF16 = mybir.dt.bfloat16
F8 = mybir.dt.float8e4
I32 = mybir.dt.int32
WSCALE = 16.0  # fp8 weight pre-scale (subnormal coverage); undone at drains

B, C, L = 8, 512, 2048
G = 4            # groupnorm groups; group size 128 == one partition tile
NH, HD = 4, 128  # heads, head dim
CT = C // 128    # 4 channel tiles
LC = L // 512    # 4 l-chunks of 512
LT = L // 128    # 16 l-tiles of 128
NG = LT // 2     # 8 score groups of 2 k-tiles
EPS = 1e-6
SM_SCALE = float(HD) ** -0.5
ESHIFT = -3.0    # exp(s*scale + ESHIFT): keeps e within fp8e4 range
RSQRT_MAGIC = 0x5F3759DF
# Schraudolph fast-exp: bitcast_f32(int(A*s + B)) ~= exp(s*SM_SCALE+ESHIFT)
SCH_A = (8388608.0 / 0.6931471805599453) * SM_SCALE
SCH_B = 1064866805.0 + ESHIFT * (8388608.0 / 0.6931471805599453)

AFT = mybir.ActivationFunctionType
ALU = mybir.AluOpType
DR = mybir.MatmulPerfMode.DoubleRow


def build_attn_block(nc):
    x_d = nc.dram_tensor("x", [C, L], F32, kind="ExternalInput").ap()
    gs_d = nc.dram_tensor("gn_scale", [C], F32, kind="ExternalInput").ap()
    gb_d = nc.dram_tensor("gn_bias", [C], F32, kind="ExternalInput").ap()
    w_d = {}
    b_d = {}
    for nm in ("q", "k", "v", "o"):
        w_d[nm] = nc.dram_tensor(f"w{nm}", [C, C], F32, kind="ExternalInput").ap()
        b_d[nm] = nc.dram_tensor(f"b{nm}", [C], F32, kind="ExternalInput").ap()
    out_d = nc.dram_tensor("out", [C, L], F32, kind="ExternalOutput").ap()

    with tile.TileContext(nc) as tc:
        with (
            tc.tile_pool(name="const", bufs=1) as const,
            tc.tile_pool(name="wstage", bufs=1) as wstage,
            tc.tile_pool(name="w8", bufs=1) as w8pool,
            tc.tile_pool(name="wt", bufs=1) as wt,
            tc.tile_pool(name="big", bufs=1) as big,
            tc.tile_pool(name="small", bufs=6) as small,
            tc.tile_pool(name="epool", bufs=16) as epool,
            tc.tile_pool(name="cpool", bufs=3) as cpool,
            tc.tile_pool(name="psum", bufs=1, space="PSUM") as psum,
        ):
            # ---- constants ----
            identity = const.tile([128, 128], F32)
            make_identity(nc, identity)
            ones = const.tile([128, 128], F32)
            nc.vector.memset(ones, 1.0)
            ones8_2 = const.tile([128, 2, 128], F8)  # DR lhsT for den
            nc.vector.memset(ones8_2, 1.0)
            sixt_t = const.tile([128, 1], F32)
            nc.vector.memset(sixt_t, 1.0 / WSCALE)
            esh_t = const.tile([128, 1], F32)
            nc.vector.memset(esh_t, ESHIFT)

            # ---- big persistent tiles ----
            x_sb = big.tile([128, CT, L], F32, tag="x_sb")
            h_sb = big.tile([128, CT, L], F8, tag="h_sb")
            q_sb = big.tile([128, NH, L], BF16, tag="q_sb")
            k_sb = big.tile([128, NH, L], BF16, tag="k_sb")
            vT_sb = big.tile([128, LT, C], F8, tag="vT_sb")
            attn_sb = big.tile([128, NH, L], BF16, tag="attn_sb")

            # ---- PSUM banks (manual layout) ----
            sAB = [
                psum.tile([128, 1024], F32, tag="sA", name="sA"),
                psum.tile([128, 1024], F32, tag="sB", name="sB"),
            ]
            av = [
                psum.tile([128, 512], F32, tag="av0", name="av0"),
                psum.tile([128, 512], F32, tag="av1", name="av1"),
            ]
            dn = [
                psum.tile([128, 512], F32, tag="dn0", name="dn0"),
                psum.tile([128, 512], F32, tag="dn1", name="dn1"),
            ]

            sch = w8pool.tile([128, 1024], I32, tag="sch", name="sch")

            # ---- weight tiles ----
            wts = {}
            for nm in ("q", "k", "v"):
                wts[nm] = wt.tile([128, CT, C], F8, name=f"w{nm}t")
            wts["o"] = wt.tile([128, CT, C], BF16, name="wot")

            # ================= DMA schedule =================
            x_r = x_d.rearrange("(t p) l -> p t l", p=128)

            def load_cvec(name, ap_1d):
                t = const.tile([128, CT], F32, name=name)
                nc.gpsimd.dma_start(
                    out=t, in_=ap_1d.rearrange("(t p) -> p t", p=128)
                )
                return t

            gs_sb = load_cvec("gs_sb", gs_d)
            gb_sb = load_cvec("gb_sb", gb_d)
            bq_sb = load_cvec("bq_sb", b_d["q"])
            bk_sb = load_cvec("bk_sb", b_d["k"])
            bo_sb = load_cvec("bo_sb", b_d["o"])

            stg = {}

            def stage_dma(eng, nm, obs, name):
                t = wstage.tile([128, len(obs), C], F32, tag=name, name=name)
                eng.dma_start(
                    out=t,
                    in_=w_d[nm][obs[0] * 128 : (obs[-1] + 1) * 128, :].rearrange(
                        "(o p) c -> p o c", p=128
                    ),
                )
                stg[name] = t
                return t

            # qSP : cvecs x0a x1a wk-b0 wq-b0 x2a x3[0:512] | wv01 tps ...
            # qACT: x0b x1b x2b x3[512:1024] x3[1024:1536] x3[1536:2048]
            for ct in range(2):
                nc.sync.dma_start(out=x_sb[:, ct, 0:1024], in_=x_r[:, ct, 0:1024])
                nc.scalar.dma_start(
                    out=x_sb[:, ct, 1024:2048], in_=x_r[:, ct, 1024:2048]
                )
            stage_dma(nc.sync, "k", (0,), "kb0")
            stage_dma(nc.sync, "q", (0,), "qb0")
            nc.sync.dma_start(out=x_sb[:, 2, 0:1024], in_=x_r[:, 2, 0:1024])
            nc.scalar.dma_start(out=x_sb[:, 2, 1024:2048], in_=x_r[:, 2, 1024:2048])
            nc.sync.dma_start(out=x_sb[:, 3, 0:512], in_=x_r[:, 3, 0:512])
            nc.scalar.dma_start(out=x_sb[:, 3, 512:1024], in_=x_r[:, 3, 512:1024])
            nc.scalar.dma_start(out=x_sb[:, 3, 1024:1536], in_=x_r[:, 3, 1024:1536])
            nc.scalar.dma_start(out=x_sb[:, 3, 1536:2048], in_=x_r[:, 3, 1536:2048])
            stage_dma(nc.sync, "v", (0, 1), "v01")
            stage_dma(nc.sync, "o", (0, 1), "o01")

            bv_bc = const.tile([128, C], F32)  # bv broadcast across partitions
            # SWDGE weight stream: gate the first transfer behind the x
            # stream's last piece (tiny GPSIMD marker copy creates the WAW)
            stg_v23 = wstage.tile([128, 2, C], F32, tag="v23", name="v23")
            stg["v23"] = stg_v23
            nc.gpsimd.tensor_copy(
                stg_v23[:, 0, 0:1], x_sb[:, 3, 2047:2048]
            )
            nc.gpsimd.dma_start(
                out=stg_v23,
                in_=w_d["v"][2 * 128 : 4 * 128, :].rearrange(
                    "(o p) c -> p o c", p=128
                ),
            )
            nc.gpsimd.dma_start(
                out=bv_bc,
                in_=bass.AP(
                    tensor=b_d["v"].tensor,
                    offset=b_d["v"].offset,
                    ap=[[0, 128]] + list(b_d["v"].ap),
                ),
            )
            stage_dma(nc.gpsimd, "k", (1, 2), "k12")
            stage_dma(nc.gpsimd, "k", (3,), "k3")
            stage_dma(nc.gpsimd, "q", (1,), "q1")
            stage_dma(nc.gpsimd, "q", (2,), "q2")
            stage_dma(nc.gpsimd, "q", (3,), "q3")
            stage_dma(nc.gpsimd, "o", (2, 3), "o23")

            # ---- weight transposes: f32 PE transposes from stage into a
            # transient PSUM bank, drained (x16 for fp8 targets) by DVE ----
            def xpose_ot(nm, name, i, ot, bank):
                pt = bank[:, 0:512]
                for ctp in range(CT):
                    nc.tensor.transpose(
                        pt[:, ctp * 128 : (ctp + 1) * 128],
                        stg[name][:, i, ctp * 128 : (ctp + 1) * 128],
                        identity,
                    )
                dst = wts[nm][:, :, ot * 128 : (ot + 1) * 128]
                pr = pt.rearrange("p (c t) -> p c t", c=CT)
                if nm == "o":
                    nc.vector.tensor_copy(dst, pr)
                else:
                    nc.vector.tensor_scalar_mul(dst, pr, WSCALE)

            # ================= groupnorm =================
            ab_t = {}

            def emit_gn_stats(ct, chunks):
                stats = small.tile([128, len(chunks), 6], F32, tag="stats")
                for i, (lo, hi) in enumerate(chunks):
                    nc.vector.bn_stats(out=stats[:, i, :], in_=x_sb[:, ct, lo:hi])
                mv = small.tile([128, 2], F32, tag="mv")
                nc.vector.bn_aggr(out=mv, in_=stats)
                stat2 = small.tile([128, 2], F32, tag="stat2")
                nc.vector.tensor_copy(stat2[:, 0:1], mv[:, 0:1])
                nc.vector.scalar_tensor_tensor(
                    out=stat2[:, 1:2],
                    in0=mv[:, 0:1],
                    scalar=mv[:, 0:1],
                    in1=mv[:, 1:2],
                    op0=ALU.mult,
                    op1=ALU.add,
                )
                pg = dn[ct % 2][:, 0:2]
                nc.tensor.matmul(pg, ones, stat2, start=True, stop=True)
                mean_t = small.tile([128, 1], F32, tag="mean_t")
                nc.vector.tensor_scalar_mul(mean_t, pg[:, 0:1], 1.0 / 128.0)
                var_t = small.tile([128, 1], F32, tag="var_t")
                nc.vector.tensor_scalar_mul(var_t, pg[:, 1:2], 1.0 / 128.0)
                mm_t = small.tile([128, 1], F32, tag="mm_t")
                nc.vector.tensor_mul(mm_t, mean_t, mean_t)
                nc.vector.tensor_sub(var_t, var_t, mm_t)
                nc.vector.tensor_scalar_add(var_t, var_t, EPS)
                # rstd = 1/sqrt(var): bit-trick + 2 Newton iterations
                yi = small.tile([128, 1], I32, tag="yi")
                nc.vector.tensor_scalar(
                    out=yi,
                    in0=var_t.bitcast(I32),
                    scalar1=1,
                    scalar2=None,
                    op0=ALU.logical_shift_right,
                )
                nc.vector.tensor_scalar(
                    out=yi,
                    in0=yi,
                    scalar1=-1,
                    scalar2=RSQRT_MAGIC,
                    op0=ALU.mult,
                    op1=ALU.add,
                )
                y = yi.bitcast(F32)
                for _ in range(2):
                    t1 = small.tile([128, 1], F32, tag="t1")
                    nc.vector.tensor_mul(t1, y, y)
                    nc.vector.tensor_mul(t1, t1, var_t)
                    nc.vector.tensor_scalar(
                        out=t1,
                        in0=t1,
                        scalar1=-0.5,
                        scalar2=1.5,
                        op0=ALU.mult,
                        op1=ALU.add,
                    )
                    nc.vector.tensor_mul(y, y, t1)
                a_t = small.tile([128, 1], F32, tag="a_t", bufs=CT)
                nc.vector.tensor_mul(a_t, y, gs_sb[:, ct : ct + 1])
                b_t = small.tile([128, 1], F32, tag="b_t", bufs=CT)
                nc.vector.tensor_mul(b_t, mean_t, a_t)
                nc.vector.tensor_sub(b_t, gb_sb[:, ct : ct + 1], b_t)
                ab_t[ct] = (a_t, b_t)
                return a_t, b_t

            def apply_act(ct, lo, hi):
                a_t, b_t = ab_t[ct]
                nc.scalar.activation(
                    h_sb[:, ct, lo:hi], x_sb[:, ct, lo:hi], AFT.Identity,
                    bias=b_t, scale=a_t,
                )

            def apply_dve(ct, lo, hi):
                a_t, b_t = ab_t[ct]
                nc.vector.tensor_scalar(
                    out=h_sb[:, ct, lo:hi], in0=x_sb[:, ct, lo:hi],
                    scalar1=a_t, scalar2=b_t, op0=ALU.mult, op1=ALU.add,
                )

            for ct in range(2):
                emit_gn_stats(ct, [(i * 512, (i + 1) * 512) for i in range(4)])
                apply_act(ct, 0, 1024)
                apply_act(ct, 1024, 2048)

            # ---- b0 transposes (f32, PE idle anyway) + k0/q0 p2=0 ----
            def xpose_b0(name, nm):
                pt = dn[0 if nm == "k" else 1][:, 0:512]
                for ctp in range(CT):
                    nc.tensor.transpose(
                        pt[:, ctp * 128 : (ctp + 1) * 128],
                        stg[name][:, 0, ctp * 128 : (ctp + 1) * 128],
                        identity,
                    )
                nc.vector.tensor_scalar_mul(
                    wts[nm][:, :, 0:128],
                    pt.rearrange("p (c t) -> p c t", c=CT),
                    WSCALE,
                )

            xpose_b0("kb0", "k")
            xpose_b0("qb0", "q")

            def k0_mm(lc, p2, start, stop):
                nc.tensor.matmul(
                    sAB[lc // 2][:, (lc % 2) * 512 : (lc % 2 + 1) * 512],
                    wts["k"][:, 2 * p2 : 2 * p2 + 2, 0:128],
                    h_sb[:, 2 * p2 : 2 * p2 + 2, lc * 512 : (lc + 1) * 512],
                    start=start, stop=stop, perf_mode=DR,
                )

            def q0_mm(lc, bank, p2, start, stop):
                nc.tensor.matmul(
                    bank,
                    wts["q"][:, 2 * p2 : 2 * p2 + 2, 0:128],
                    h_sb[:, 2 * p2 : 2 * p2 + 2, lc * 512 : (lc + 1) * 512],
                    start=start, stop=stop, perf_mode=DR,
                )

            def k0_ct(lc, ct, start, stop):
                nc.tensor.matmul(
                    sAB[lc // 2][:, (lc % 2) * 512 : (lc % 2 + 1) * 512],
                    wts["k"][:, ct, 0:128],
                    h_sb[:, ct, lc * 512 : (lc + 1) * 512],
                    start=start, stop=stop,
                )

            def q0_ct(lc, bank, ct, start, stop):
                nc.tensor.matmul(
                    bank,
                    wts["q"][:, ct, 0:128],
                    h_sb[:, ct, lc * 512 : (lc + 1) * 512],
                    start=start, stop=stop,
                )

            for ct in range(2):
                for lc in range(LC):
                    k0_ct(lc, ct, ct == 0, False)
                q0_ct(0, av[0][:, 0:512], ct, ct == 0, False)
                q0_ct(1, av[1][:, 0:512], ct, ct == 0, False)

            emit_gn_stats(2, [(i * 512, (i + 1) * 512) for i in range(4)])
            apply_act(2, 0, 1024)
            apply_act(2, 1024, 2048)
            for lc in range(LC):
                k0_ct(lc, 2, False, False)
            q0_ct(0, av[0][:, 0:512], 2, False, False)
            q0_ct(1, av[1][:, 0:512], 2, False, False)

            # ct3: quarters land staggered (b, c, d on qACT; a last on qSP)
            emit_gn_stats(
                3, [(512, 1024), (1024, 1536), (1536, 2048), (0, 512)]
            )
            apply_act(3, 0, 512)
            apply_dve(3, 512, 1024)

            def drain_kq(dst, h, lc, src, bias, eng):
                if eng == "act":
                    nc.scalar.activation(
                        dst[:, h, lc * 512 : (lc + 1) * 512], src, AFT.Identity,
                        bias=bias[:, h : h + 1], scale=1.0 / WSCALE,
                    )
                else:
                    nc.vector.tensor_scalar(
                        out=dst[:, h, lc * 512 : (lc + 1) * 512], in0=src,
                        scalar1=1.0 / WSCALE, scalar2=bias[:, h : h + 1],
                        op0=ALU.mult, op1=ALU.add,
                    )

            # k0 must fully finish AND drain out of sA/sB before unit 0's
            # score matmuls reuse those banks. Per-ct accumulation means
            # each ct3-plane matmul waits only on its own apply slice.
            k0_ct(0, 3, False, True)
            q0_ct(0, av[0][:, 0:512], 3, False, True)
            drain_kq(k_sb, 0, 0, sAB[0][:, 0:512], bk_sb, "dve")
            drain_kq(q_sb, 0, 0, av[0][:, 0:512], bq_sb, "act")
            k0_ct(1, 3, False, True)
            drain_kq(k_sb, 0, 1, sAB[0][:, 512:1024], bk_sb, "dve")
            apply_act(3, 1024, 2048)
            k0_ct(2, 3, False, True)
            k0_ct(3, 3, False, True)
            drain_kq(k_sb, 0, 2, sAB[1][:, 0:512], bk_sb, "dve")
            drain_kq(k_sb, 0, 3, sAB[1][:, 512:1024], bk_sb, "dve")

            # ================= attention building blocks =================
            def emit_qk(h, qc, g, dve=False):
                ps = sAB[g % 2]
                for j in range(2):
                    kt = 2 * g + j
                    nc.tensor.matmul(
                        ps[:, j * 512 : (j + 1) * 512],
                        k_sb[:, h, kt * 128 : (kt + 1) * 128],
                        q_sb[:, h, qc * 512 : (qc + 1) * 512],
                        start=True, stop=True,
                    )
                e = epool.tile([128, 1024], F8, tag="e", name="e")
                if dve:
                    # Schraudolph fast-exp on DVE: offloads ACT (the pacer)
                    nc.vector.tensor_scalar(
                        out=sch, in0=ps, scalar1=SCH_A, scalar2=SCH_B,
                        op0=ALU.mult, op1=ALU.add,
                    )
                    nc.vector.tensor_copy(e, sch.bitcast(F32))
                else:
                    nc.scalar.activation(
                        e, ps, AFT.Exp, bias=esh_t, scale=SM_SCALE
                    )
                return e

            def e3d(e):
                return e.rearrange("p (j q) -> p j q", j=2)

            def emit_pden(pden, e, g):
                nc.tensor.matmul(
                    pden, ones8_2, e3d(e),
                    start=(g == 0), stop=(g == NG - 1), perf_mode=DR,
                )

            def emit_av(h, pav, e, g):
                nc.tensor.matmul(
                    pav,
                    vT_sb[:, 2 * g : 2 * g + 2, h * 128 : (h + 1) * 128],
                    e3d(e),
                    start=(g == 0), stop=(g == NG - 1), perf_mode=DR,
                )

            def finish_unit(st):
                h, qc, pav, pden = st
                rden = cpool.tile([128, 512], F32, tag="rden", name="rden")
                nc.vector.reciprocal_approx_fast(rden, pden)
                nc.vector.tensor_mul(
                    attn_sb[:, h, qc * 512 : (qc + 1) * 512], pav, rden
                )

            def emit_proj_group(h, i, bank):
                # i in 0..7: 0-3 -> k lc=i, 4-7 -> q lc=i-4
                dst, wtt, bias = (
                    (k_sb, wts["k"], bk_sb) if i < 4 else (q_sb, wts["q"], bq_sb)
                )
                lc = i % 4
                pp = bank[:, 0:512]
                for p2 in range(2):
                    nc.tensor.matmul(
                        pp,
                        wtt[:, 2 * p2 : 2 * p2 + 2, h * 128 : (h + 1) * 128],
                        h_sb[:, 2 * p2 : 2 * p2 + 2, lc * 512 : (lc + 1) * 512],
                        start=(p2 == 0), stop=(p2 == 1), perf_mode=DR,
                    )
                drain_kq(dst, h, lc, pp, bias, "dve")

            def q0_late(lc, bank):
                pp = bank[:, 0:512]
                for p2 in range(2):
                    q0_mm(lc, pp, p2, p2 == 0, p2 == 1)

            def emit_v_tile(lt, bank):
                pv = bank[:, 0:512]
                for p2 in range(2):
                    nc.tensor.matmul(
                        pv,
                        h_sb[:, 2 * p2 : 2 * p2 + 2, lt * 128 : (lt + 1) * 128],
                        wts["v"][:, 2 * p2 : 2 * p2 + 2, :],
                        start=(p2 == 0), stop=(p2 == 1), perf_mode=DR,
                    )
                nc.vector.scalar_tensor_tensor(
                    out=vT_sb[:, lt, :], in0=pv, scalar=sixt_t, in1=bv_bc,
                    op0=ALU.mult, op1=ALU.add,
                )

            dmae = [nc.sync, nc.scalar, nc.gpsimd, nc.scalar]

            def emit_out_proj_ot(qc, ot, bank, nq):
                pop = bank[:, 0:512]
                for ctp in range(CT):
                    nc.tensor.matmul(
                        pop,
                        wts["o"][:, ctp, ot * 128 : (ot + 1) * 128],
                        attn_sb[:, ctp, qc * 512 : (qc + 1) * 512],
                        start=(ctp == 0), stop=(ctp == CT - 1),
                    )
                ot_sb = cpool.tile([128, 512], F32, tag="ot_sb")
                nc.vector.scalar_tensor_tensor(
                    out=ot_sb, in0=pop, scalar=bo_sb[:, ot : ot + 1],
                    in1=x_sb[:, ot, qc * 512 : (qc + 1) * 512],
                    op0=ALU.add, op1=ALU.add,
                )
                dmae[nq].dma_start(
                    out=out_d[ot * 128 : (ot + 1) * 128, qc * 512 : (qc + 1) * 512],
                    in_=ot_sb,
                )

            # ================= unit schedule =================
            sched = [
                (0, 0), (0, 1), (0, 2), (1, 0), (1, 1), (2, 0), (0, 3),
                (1, 2), (2, 1), (3, 0), (1, 3), (2, 2), (3, 1), (3, 2),
                (2, 3), (3, 3),
            ]

            jobs = {}

            def add_job(idx, g, fn):
                jobs.setdefault((idx, g), []).append(fn)

            def mk(fn, *a, **kw):
                return lambda: fn(*a, **kw)

            # --- u0: q0 rest (dn0/av0) + wv transposes (dn1/av1) ---
            add_job(0, 1, mk(q0_ct, 1, av[1][:, 0:512], 3, False, True))
            add_job(0, 2, mk(q0_late, 2, dn[0]))
            add_job(0, 3, mk(q0_late, 3, av[0]))
            add_job(0, 3, mk(drain_kq, q_sb, 0, 1, av[1][:, 0:512], bq_sb, "dve"))
            add_job(0, 4, mk(xpose_ot, "v", "v01", 0, 0, dn[1]))
            add_job(0, 4, mk(drain_kq, q_sb, 0, 2, dn[0][:, 0:512], bq_sb, "dve"))
            add_job(0, 5, mk(xpose_ot, "v", "v01", 1, 1, av[1]))
            add_job(0, 5, mk(drain_kq, q_sb, 0, 3, av[0][:, 0:512], bq_sb, "dve"))
            add_job(0, 6, mk(xpose_ot, "v", "v23", 0, 2, dn[1]))
            add_job(0, 7, mk(xpose_ot, "v", "v23", 1, 3, av[1]))
            # --- u1: v tiles + k1/q1 transposes (transients dn1/av1) ---
            for g, lts in ((1, range(0, 6)), (2, range(6, 12)), (3, range(12, 16))):
                for lt in lts:
                    add_job(1, g, mk(emit_v_tile, lt,
                                     dn[1] if lt % 2 else av[1]))
            add_job(1, 5, mk(xpose_ot, "k", "k12", 0, 1, dn[1]))
            add_job(1, 7, mk(xpose_ot, "q", "q1", 0, 1, av[1]))
            # --- u2: k1/q1 projections (transients dn0/av0) ---
            add_job(2, 3, mk(emit_proj_group, 1, 0, av[0]))
            add_job(2, 4, mk(emit_proj_group, 1, 1, dn[0]))
            add_job(2, 5, mk(emit_proj_group, 1, 4, av[0]))
            add_job(2, 6, mk(emit_proj_group, 1, 2, dn[0]))
            add_job(2, 7, mk(emit_proj_group, 1, 3, av[0]))
            add_job(2, 8, mk(xpose_ot, "k", "k12", 1, 2, dn[0]))
            # --- u3: k3/q2 transposes + q1 rest + k2 start (dn1/av1) ---
            add_job(3, 3, mk(xpose_ot, "k", "k3", 0, 3, dn[1]))
            add_job(3, 4, mk(emit_proj_group, 1, 5, av[1]))
            add_job(3, 5, mk(emit_proj_group, 1, 6, dn[1]))
            add_job(3, 6, mk(emit_proj_group, 1, 7, av[1]))
            add_job(3, 7, mk(xpose_ot, "q", "q2", 0, 2, dn[1]))
            add_job(3, 8, mk(emit_proj_group, 2, 0, av[1]))
            # --- u4: k2 rest + q2-lc0/lc1 + q3 transpose (dn0/av0) ---
            add_job(4, 3, mk(emit_proj_group, 2, 1, dn[0]))
            add_job(4, 4, mk(emit_proj_group, 2, 2, av[0]))
            add_job(4, 5, mk(emit_proj_group, 2, 3, dn[0]))
            add_job(4, 6, mk(xpose_ot, "q", "q3", 0, 3, av[0]))
            add_job(4, 7, mk(emit_proj_group, 2, 4, dn[0]))
            add_job(4, 8, mk(emit_proj_group, 2, 5, av[0]))
            # --- u5: q2 rest + k3 projections (dn1/av1) ---
            add_job(5, 3, mk(emit_proj_group, 2, 6, dn[1]))
            add_job(5, 4, mk(emit_proj_group, 2, 7, av[1]))
            add_job(5, 5, mk(emit_proj_group, 3, 0, dn[1]))
            add_job(5, 6, mk(emit_proj_group, 3, 1, av[1]))
            add_job(5, 7, mk(emit_proj_group, 3, 2, dn[1]))
            add_job(5, 8, mk(emit_proj_group, 3, 3, av[1]))
            # --- u6: q3 projections + wo transposes start (dn0/av0) ---
            add_job(6, 3, mk(emit_proj_group, 3, 4, dn[0]))
            add_job(6, 4, mk(emit_proj_group, 3, 5, av[0]))
            add_job(6, 5, mk(emit_proj_group, 3, 6, dn[0]))
            add_job(6, 6, mk(emit_proj_group, 3, 7, av[0]))
            add_job(6, 7, mk(xpose_ot, "o", "o01", 0, 0, dn[0]))
            add_job(6, 8, mk(xpose_ot, "o", "o01", 1, 1, av[0]))
            # --- u7: wo rest (dn1/av1) ---
            add_job(7, 3, mk(xpose_ot, "o", "o23", 0, 2, dn[1]))
            add_job(7, 5, mk(xpose_ot, "o", "o23", 1, 3, av[1]))
            # --- out-projections: qc0 @ u11/u12, qc1 @ u14, qc2 @ u15 ---
            pops = [
                (11, 4, 0, 0), (11, 6, 0, 1), (12, 4, 0, 2), (12, 6, 0, 3),
                (14, 4, 1, 0), (14, 5, 1, 1), (14, 6, 1, 2), (14, 7, 1, 3),
                (15, 4, 2, 0), (15, 5, 2, 1), (15, 6, 2, 2), (15, 7, 2, 3),
            ]
            for n, (idx, g, qc, ot) in enumerate(pops):
                bank = dn[idx % 2] if n % 2 == 0 else av[idx % 2]
                add_job(idx, g, mk(emit_out_proj_ot, qc, ot, bank, n % 2))

            # ================= main unit loop =================
            st = [None] * 16
            e_of = [None] * 16
            for idx, (h, qc) in enumerate(sched):
                pden = dn[idx % 2][:, 0:512]
                pav = av[idx % 2][:, 0:512]
                st[idx] = (h, qc, pav, pden)
                es = []
                prev = st[idx - 1] if idx >= 1 else None
                pes = e_of[idx - 1] if idx >= 1 else None
                es.append(emit_qk(h, qc, 0))
                for g in range(1, NG + 1):
                    if g < NG:
                        es.append(emit_qk(h, qc, g,
                                          dve=(g == 7 and 6 <= idx <= 13)))
                    # accumulate previous unit's den first (completes right
                    # after its source exp), then its AV (u1: lag 3 behind
                    # the v tiles)
                    if prev is not None:
                        ph, _, ppav, ppden = prev
                        emit_pden(ppden, pes[g - 1], g - 1)
                        lag = 2 if idx == 1 else 0
                        gg = g - 1 - lag
                        if gg >= 0:
                            emit_av(ph, ppav, pes[gg], gg)
                    for fn in jobs.get((idx, g), []):
                        fn()
                    if g == 2 and idx >= 2 and st[idx - 2] is not None:
                        finish_unit(st[idx - 2])
                        st[idx - 2] = None
                if prev is not None and idx == 1:
                    ph, _, ppav, _ = prev
                    for gg in range(NG - 2, NG):
                        emit_av(ph, ppav, pes[gg], gg)
                e_of[idx] = es

            # ================= tail =================
            # u15's den/av accumulate into the freed score banks; qc3
            # out-projections stream on 4 parallel DMA queues
            h15, qc15 = sched[15]
            pden15 = sAB[0][:, 0:512]
            pav15 = sAB[0][:, 512:1024]
            finish_unit(st[14])
            for g in range(NG):
                nc.tensor.matmul(
                    pden15, ones8_2, e3d(e_of[15][g]),
                    start=(g == 0), stop=(g == NG - 1), perf_mode=DR,
                )
            rden15 = cpool.tile([128, 512], F32, tag="rden", name="rden15")
            nc.vector.reciprocal_approx_fast(rden15, pden15)
            for g in range(NG):
                nc.tensor.matmul(
                    pav15,
                    vT_sb[:, 2 * g : 2 * g + 2, h15 * 128 : (h15 + 1) * 128],
                    e3d(e_of[15][g]),
                    start=(g == 0), stop=(g == NG - 1), perf_mode=DR,
                )
            nc.vector.tensor_mul(
                attn_sb[:, h15, qc15 * 512 : (qc15 + 1) * 512], pav15, rden15
            )
            for ot in range(CT):
                bank = [sAB[1][:, 0:512], sAB[1][:, 512:1024],
                        dn[0][:, 0:512], av[0][:, 0:512]][ot]
                for ctp in range(CT):
                    nc.tensor.matmul(
                        bank,
                        wts["o"][:, ctp, ot * 128 : (ot + 1) * 128],
                        attn_sb[:, ctp, 3 * 512 : 4 * 512],
                        start=(ctp == 0), stop=(ctp == CT - 1),
                    )
                ot_sb = cpool.tile([128, 512], F32, tag="ot_sb")
                nc.vector.scalar_tensor_tensor(
                    out=ot_sb, in0=bank, scalar=bo_sb[:, ot : ot + 1],
                    in1=x_sb[:, ot, 3 * 512 : 4 * 512],
                    op0=ALU.add, op1=ALU.add,
                )
                dmae[[0, 1, 2, 0][ot]].dma_start(
                    out=out_d[ot * 128 : (ot + 1) * 128, 3 * 512 : 4 * 512],
                    in_=ot_sb,
                )
    nc.compile()
    return nc


_NC_CACHE = {}


def _get_nc():
    if "nc" not in _NC_CACHE:
        nc = bacc.Bacc("TRN2", debug=False)
        build_attn_block(nc)
        _NC_CACHE["nc"] = nc
    return _NC_CACHE["nc"]


def run(trace=False, **inputs):
    nc = _get_nc()
    xs = np.ascontiguousarray(np.asarray(inputs["x"], dtype=np.float32))
    shared = {}
    for nm in ("gn_scale", "gn_bias", "wq", "bq", "wk", "bk", "wv", "bv", "wo", "bo"):
        shared[nm] = np.ascontiguousarray(np.asarray(inputs[nm], dtype=np.float32))
    in_maps = [dict(shared, x=xs[b]) for b in range(B)]
    res = run_bass_kernel_spmd(nc, in_maps, core_ids=list(range(B)), trace=trace)
    out = np.stack([res.results[b]["out"] for b in range(B)], axis=0)
    return out, res


def kernel(**inputs):
    out, _ = run(trace=bool(os.environ.get("ATTN_TRACE")), **inputs)
    return out
